# revision 10
# baseline (speedup 1.0000x reference)
"""Trainium2 Bass kernel for nn_EuclidDistance_Assign_Module (vq_codebook).

LayerNorm -> euclidean distance to 256 codebook vectors -> argmin labels,
softmax(-32*d) assignment, assignment @ codebook reconstruction, plus
codebook self-distance matrix.

Data-parallel over batch B=8 across 8 NeuronCores. Each core handles
16384 tokens (C=512, K=256) independently; cluster_center is replicated.

Self-contained: hardcodes shapes from the problem spec.
"""

import os
import sys

import numpy as np

sys.path.insert(0, "/opt/trn_rl_repo")

import json  # noqa: E402
import types  # noqa: E402

import concourse.bass as bass  # noqa: E402
import concourse.tile as tile  # noqa: E402
from concourse import masks, mybir  # noqa: E402
from concourse.bass_utils import run_bass_kernel_spmd  # noqa: E402


def _split_excess_waits(bir: bytes, max_waits: int = 1) -> bytes:
    """This container's walrus build allows only one sync-wait per
    instruction; Tile emits several. Split the excess onto Drain carriers
    inserted just before the over-waiting instruction (same engine, so the
    waits still complete before it issues)."""
    d = json.loads(bir)
    for f in d["functions"]:
        for bb in f["blocks"]:
            new_insts = []
            ctr = 0
            for inst in bb["instructions"]:
                si = inst.get("sync_info") or {}
                ow = si.get("on_wait") or []
                if len(ow) > max_waits:
                    excess, keep = ow[:-max_waits], ow[-max_waits:]
                    for i in range(0, len(excess), max_waits):
                        ctr += 1
                        new_insts.append({
                            "name": f"{inst['name']}-ws{ctr}",
                            "opcode": "Drain",
                            "engine": inst["engine"],
                            "debug": inst.get("debug", 0),
                            "ins": [], "outs": [],
                            "sync_info": {"on_update": [],
                                          "on_wait": excess[i:i + max_waits]},
                        })
                    si["on_wait"] = keep
                    inst["sync_info"] = si
                new_insts.append(inst)
            bb["instructions"] = new_insts
    return json.dumps(d).encode()

F32 = mybir.dt.float32
I32 = mybir.dt.int32
AF = mybir.ActivationFunctionType
ALU = mybir.AluOpType
AX = mybir.AxisListType

N_CORES = 8
P = 128            # partitions / tokens per tile
C = 512            # channels
K = 256            # codebook size
N_TOK = 16384      # tokens per core (= 16*32*32)
NT = N_TOK // P    # 128 tiles per core
ALPHA = 32.0
LN_EPS = 1e-5

_CACHE = {}


def _build_program():
    nc = bass.Bass("TRN2", target_bir_lowering=False, debug=False)

    x_d = nc.dram_tensor("x", [N_TOK, C], F32, kind="ExternalInput").ap()
    cc_d = nc.dram_tensor("cluster_center", [K, C], F32, kind="ExternalInput").ap()

    dist_d = nc.dram_tensor("x_distance", [N_TOK, K], F32, kind="ExternalOutput").ap()
    asgn_d = nc.dram_tensor("x_assign", [N_TOK, K], F32, kind="ExternalOutput").ap()
    cdist_d = nc.dram_tensor("cluster_dist", [K, K], F32, kind="ExternalOutput").ap()
    xrec_d = nc.dram_tensor("x_rec", [N_TOK, C], F32, kind="ExternalOutput").ap()
    feat_d = nc.dram_tensor("feature", [N_TOK, C], F32, kind="ExternalOutput").ap()
    lbl_d = nc.dram_tensor("feature_label", [N_TOK], I32, kind="ExternalOutput").ap()

    from contextlib import ExitStack

    with tile.TileContext(nc) as tc:
        with ExitStack() as ctx:
            const = ctx.enter_context(tc.tile_pool(name="const", bufs=1))
            psum = ctx.enter_context(tc.tile_pool(name="psum", bufs=2, space="PSUM"))
            p_big = ctx.enter_context(tc.tile_pool(name="big", bufs=4))
            p_mid = ctx.enter_context(tc.tile_pool(name="mid", bufs=4))
            p_sm = ctx.enter_context(tc.tile_pool(name="small", bufs=6))

            # ---------------- constants ----------------
            ident = const.tile([P, P], F32)
            masks.make_identity(nc, ident[:])

            ones1 = const.tile([1, P], F32)
            nc.gpsimd.memset(ones1[:], 1.0)

            eps_t = const.tile([P, 1], F32)
            nc.gpsimd.memset(eps_t[:], LN_EPS)

            # codebook rows: 2 tiles [128, 512]
            cc_sb = []
            for m in range(2):
                t = const.tile([P, C], F32, tag=f"cc{m}")
                nc.sync.dma_start(t[:], cc_d[m * P:(m + 1) * P, :])
                cc_sb.append(t)

            # |c_k|^2 as column pair [128, 2] (chunk m in column m)
            c2cp = const.tile([P, 2], F32)
            for m in range(2):
                ccsq = p_big.tile([P, C], F32, tag="xn")
                nc.gpsimd.tensor_tensor(
                    ccsq[:], cc_sb[m][:], cc_sb[m][:], op=ALU.mult,
                )
                nc.vector.tensor_reduce(
                    c2cp[:, m:m + 1], ccsq[:], axis=AX.X, op=ALU.add,
                )

            # transpose to get |c|^2 as a [1, 256] row
            c2t_ps = psum.tile([2, P], F32, tag="mps")
            nc.tensor.transpose(c2t_ps[:], c2cp[:], ident[:])
            c2pt = const.tile([2, P], F32)
            nc.scalar.copy(c2pt[:], c2t_ps[:])
            c2row = const.tile([1, K], F32)
            nc.sync.dma_start(c2row[0:1, 0:P], c2pt[0:1, :])
            nc.sync.dma_start(c2row[0:1, P:K], c2pt[1:2, :])
            # -2 * |c|^2 row for the cluster_dist pass
            c2rowm2 = const.tile([1, K], F32)
            nc.vector.tensor_scalar_mul(c2rowm2[:], c2row[:], -2.0)

            # cT2[j] = -2 * cluster_center^T chunk j : [128(c), 256(k)]
            cT2_sb = []
            for j in range(4):
                ps = psum.tile([P, K], F32, tag="mps")
                for m in range(2):
                    nc.tensor.transpose(
                        ps[:, m * P:(m + 1) * P],
                        cc_sb[m][:, j * P:(j + 1) * P],
                        ident[:],
                    )
                t = const.tile([P, K], F32, tag=f"cT2_{j}")
                nc.scalar.mul(t[:], ps[:], -2.0)
                cT2_sb.append(t)

            # argmin helper: bigmidx[p, k] = 256 - k
            bigmidx = const.tile([P, K], F32)
            nc.gpsimd.iota(
                bigmidx[:], pattern=[[-1, K]], base=K, channel_multiplier=0,
                allow_small_or_imprecise_dtypes=True,
            )

            # per-tile labels parked here (col t), transposed+cast at the end
            lblbuf = const.tile([P, NT], F32)

            # ---------------- cluster_dist [256,256] ----------------
            # dist2[k1,k2] = |c_k1|^2 + |c_k2|^2 - 2<c_k1,c_k2>
            # matmul of cT2 x cT2 gives 4*<,>, so rescale by -0.5 and use
            # the -2*|c|^2 row via a K=1 matmul.
            for r in range(2):
                mps = psum.tile([P, K], F32, tag="mps")
                for j in range(4):
                    nc.tensor.matmul(
                        mps[:], lhsT=cT2_sb[j][:, r * P:(r + 1) * P],
                        rhs=cT2_sb[j][:], start=(j == 0), stop=False,
                    )
                nc.tensor.matmul(
                    mps[:], lhsT=ones1[:], rhs=c2rowm2[:], start=False, stop=True,
                )
                # t = max(-0.5*mps + |c_k1|^2, tiny)   [tiny: diagonal can
                # cancel to a small negative; Ln needs > 0]
                tcd = p_mid.tile([P, K], F32, tag="dist")
                nc.vector.tensor_scalar(
                    tcd[:], mps[:], -0.5, c2cp[:, r:r + 1],
                    op0=ALU.mult, op1=ALU.add,
                )
                tcd2 = p_mid.tile([P, K], F32, tag="e")
                nc.vector.tensor_scalar_max(tcd2[:], tcd[:], 1e-12)
                lncd = p_mid.tile([P, K], F32, tag="lnd")
                nc.scalar.activation(lncd[:], tcd2[:], AF.Ln)
                dcd = p_mid.tile([P, K], F32, tag="asgn")
                nc.scalar.activation(dcd[:], lncd[:], AF.Exp, scale=0.5)
                nc.sync.dma_start(cdist_d[r * P:(r + 1) * P, :], dcd[:])

            # ---------------- main loop over token tiles ----------------
            for t in range(NT):
                xt = p_big.tile([P, C], F32, tag="x")
                nc.sync.dma_start(xt[:], x_d[t * P:(t + 1) * P, :])

                # LayerNorm stats
                st6 = p_sm.tile([P, 6], F32, tag="st6")
                nc.vector.bn_stats(st6[:], xt[:])
                mv = p_sm.tile([P, 2], F32, tag="mv")
                nc.vector.bn_aggr(mv[:], st6[:])
                mean = mv[:, 0:1]
                var = mv[:, 1:2]

                # rstd = exp(-0.5*ln(var+eps));  (Rsqrt ACT is banned)
                lnv = p_sm.tile([P, 1], F32, tag="lnv")
                nc.scalar.activation(lnv[:], var, AF.Ln, bias=eps_t[:])
                rstd = p_sm.tile([P, 1], F32, tag="rstd")
                nc.scalar.activation(rstd[:], lnv[:], AF.Exp, scale=-0.5)

                # |xn|^2 = 512 * var * rstd^2 (exact: sum((x-mu)^2)*rstd^2)
                xn2a = p_sm.tile([P, 1], F32, tag="xn2a")
                nc.gpsimd.tensor_scalar(
                    xn2a[:], rstd[:], rstd[:], float(C), op0=ALU.mult, op1=ALU.mult,
                )
                xn2 = p_sm.tile([P, 1], F32, tag="xn2")
                nc.gpsimd.tensor_scalar(
                    xn2[:], xn2a[:], var, None, op0=ALU.mult,
                )
                # -mu*rstd
                nmr = p_sm.tile([P, 1], F32, tag="nmr")
                nc.gpsimd.tensor_scalar(
                    nmr[:], mean, rstd[:], -1.0, op0=ALU.mult, op1=ALU.mult,
                )

                # xn = x*rstd + (-mu*rstd)   (gamma=1, beta=0)
                xn = p_big.tile([P, C], F32, tag="xn")
                nc.gpsimd.tensor_scalar(
                    xn[:], xt[:], rstd[:], nmr[:], op0=ALU.mult, op1=ALU.add,
                )
                nc.sync.dma_start(feat_d[t * P:(t + 1) * P, :], xn[:])

                # xn^T via PE transpose, then SBUF copy for matmul weights
                xnT_ps = psum.tile([P, C], F32, tag="xnT_ps")
                for j in range(4):
                    nc.tensor.transpose(
                        xnT_ps[:, j * P:(j + 1) * P], xn[:, j * P:(j + 1) * P],
                        ident[:],
                    )
                xnT = p_big.tile([P, C], F32, tag="xnT")
                nc.scalar.copy(xnT[:], xnT_ps[:])

                # M = -2*xn.c + |c|^2   (c2row folded in as K=1 matmul)
                mps = psum.tile([P, K], F32, tag="mps")
                for j in range(4):
                    nc.tensor.matmul(
                        mps[:], lhsT=xnT[:, j * P:(j + 1) * P], rhs=cT2_sb[j][:],
                        start=(j == 0), stop=False,
                    )
                nc.tensor.matmul(
                    mps[:], lhsT=ones1[:], rhs=c2row[:], start=False, stop=True,
                )

                # dist = exp(0.5*ln(M + |xn|^2)) = sqrt(dist^2)
                lnd = p_mid.tile([P, K], F32, tag="lnd")
                nc.scalar.activation(lnd[:], mps[:], AF.Ln, bias=xn2[:])
                dist = p_mid.tile([P, K], F32, tag="dist")
                nc.scalar.activation(dist[:], lnd[:], AF.Exp, scale=0.5)
                nc.sync.dma_start(dist_d[t * P:(t + 1) * P, :], dist[:])

                # softmax(-alpha*dist) over k
                dmin = p_sm.tile([P, 1], F32, tag="dmin")
                nc.vector.tensor_reduce(dmin[:], dist[:], axis=AX.X, op=ALU.min)
                nab = p_sm.tile([P, 1], F32, tag="nab")
                nc.gpsimd.tensor_scalar(nab[:], dmin[:], ALPHA, None, op0=ALU.mult)
                e = p_mid.tile([P, K], F32, tag="e")
                esum = p_sm.tile([P, 1], F32, tag="esum")
                nc.scalar.activation(
                    e[:], dist[:], AF.Exp, bias=nab[:], scale=-ALPHA,
                    accum_out=esum[:],
                )
                res = p_sm.tile([P, 1], F32, tag="res")
                nc.vector.reciprocal(res[:], esum[:])
                asgn = p_mid.tile([P, K], F32, tag="asgn")
                nc.gpsimd.tensor_scalar(asgn[:], e[:], res[:], None, op0=ALU.mult)
                nc.sync.dma_start(asgn_d[t * P:(t + 1) * P, :], asgn[:])

                # argmin: first k where dist == dmin
                # junk = (dist == dmin) * (K - k); max over k = K - argmin
                eq = p_mid.tile([P, K], F32, tag="eq")
                nc.gpsimd.tensor_scalar(
                    eq[:], dist[:], dmin[:], None, op0=ALU.is_equal,
                )
                junk = p_mid.tile([P, K], F32, tag="junk")
                nc.gpsimd.tensor_tensor(junk[:], eq[:], bigmidx[:], op=ALU.mult)
                mx = p_sm.tile([P, 1], F32, tag="mx")
                nc.vector.tensor_reduce(mx[:], junk[:], axis=AX.X, op=ALU.max)
                nc.gpsimd.tensor_scalar(
                    lblbuf[:, t:t + 1], mx[:], -1.0, float(K),
                    op0=ALU.mult, op1=ALU.add,
                )

                # x_rec = asgn @ cc
                aT_ps = psum.tile([P, K], F32, tag="aT_ps")
                for m in range(2):
                    nc.tensor.transpose(
                        aT_ps[:, m * P:(m + 1) * P], asgn[:, m * P:(m + 1) * P],
                        ident[:],
                    )
                aT = p_mid.tile([P, K], F32, tag="aT")
                nc.vector.tensor_copy(aT[:], aT_ps[:])
                xr_ps = psum.tile([P, C], F32, tag="xr_ps")
                for m in range(2):
                    nc.tensor.matmul(
                        xr_ps[:], lhsT=aT[:, m * P:(m + 1) * P], rhs=cc_sb[m][:],
                        start=(m == 0), stop=(m == 1),
                    )
                xr = p_big.tile([P, C], F32, tag="xr")
                nc.vector.tensor_copy(xr[:], xr_ps[:])
                nc.sync.dma_start(xrec_d[t * P:(t + 1) * P, :], xr[:])

            # ---------------- labels: transpose + cast + store ----------------
            lT_ps = psum.tile([NT, P], F32, tag="xnT_ps")
            nc.tensor.transpose(lT_ps[:], lblbuf[:], ident[:])
            lT = p_big.tile([NT, P], I32, tag="lT")
            nc.scalar.copy(lT[:], lT_ps[:])
            nc.sync.dma_start(
                lbl_d.rearrange("(a b) -> a b", b=P)[:], lT[:],
            )

    orig_to_json = nc.to_json_bytes
    nc.to_json_bytes = types.MethodType(
        lambda self: _split_excess_waits(orig_to_json()), nc
    )
    return nc


def _run_numpy_fallback(x, cluster_center, ln_gamma, ln_beta):
    """General-input reference path (used only if LN params aren't 1/0)."""
    B, D, H, W, Cc = x.shape
    Kk = cluster_center.shape[0]
    xf = x.astype(np.float32)
    mu = xf.mean(-1, keepdims=True)
    var = ((xf - mu) ** 2).mean(-1, keepdims=True)
    xn = (xf - mu) / np.sqrt(var + LN_EPS) * ln_gamma + ln_beta
    x_re = xn.reshape(B, D * H * W, Cc)
    cc = cluster_center.astype(np.float32)

    def euclid(a, b):
        sq = (np.sum(a * a, -1, keepdims=True)
              + np.sum(b * b, -1)[None, :]
              - 2.0 * (a @ b.T))
        return np.sqrt(np.maximum(sq, 0.0))

    dist = np.stack([euclid(x_re[b], cc) for b in range(B)])
    feature_label = dist.argmin(-1).reshape(-1).astype(np.int32)
    x_distance = dist.reshape(B, D, H, W, Kk)
    z = -ALPHA * x_distance
    z = z - z.max(-1, keepdims=True)
    ez = np.exp(z)
    x_distance_assign = ez / ez.sum(-1, keepdims=True)
    cluster_dist = euclid(cc, cc)
    x_rec = np.einsum('bdhwk,kc->bdhwc', x_distance_assign, cc)
    feature = x_re.reshape(-1, Cc)
    return (x_distance, x_distance_assign, cluster_dist, x_rec,
            feature, feature_label)


LAST_EXEC_NS = None


def kernel(x, cluster_center, ln_gamma, ln_beta):
    global LAST_EXEC_NS
    x = np.asarray(x, dtype=np.float32)
    cluster_center = np.asarray(cluster_center, dtype=np.float32)
    ln_gamma = np.asarray(ln_gamma, dtype=np.float32)
    ln_beta = np.asarray(ln_beta, dtype=np.float32)

    if (x.shape != (8, 16, 32, 32, 512) or cluster_center.shape != (256, 512)
            or not np.all(ln_gamma == 1.0) or not np.all(ln_beta == 0.0)):
        return _run_numpy_fallback(x, cluster_center, ln_gamma, ln_beta)

    if "nc" not in _CACHE:
        _CACHE["nc"] = _build_program()
    nc = _CACHE["nc"]

    B = x.shape[0]
    x_flat = np.ascontiguousarray(x.reshape(B, N_TOK, C))
    in_maps = [
        {"x": x_flat[b], "cluster_center": cluster_center} for b in range(B)
    ]

    trace = bool(int(os.environ.get("KTRACE", "0")))
    r = run_bass_kernel_spmd(nc, in_maps, list(range(N_CORES)), trace=trace)
    LAST_EXEC_NS = r.exec_time_ns
    res = r.results

    x_distance = np.stack([res[b]["x_distance"] for b in range(B)])
    x_distance = x_distance.reshape(8, 16, 32, 32, K)
    x_assign = np.stack([res[b]["x_assign"] for b in range(B)])
    x_assign = x_assign.reshape(8, 16, 32, 32, K)
    cluster_dist = res[0]["cluster_dist"]
    x_rec = np.stack([res[b]["x_rec"] for b in range(B)])
    x_rec = x_rec.reshape(8, 16, 32, 32, C)
    feature = np.concatenate([res[b]["feature"] for b in range(B)], axis=0)
    feature_label = np.concatenate(
        [res[b]["feature_label"] for b in range(B)], axis=0
    ).astype(np.int32)

    return (x_distance, x_assign, cluster_dist, x_rec, feature, feature_label)


# revision 12
# speedup vs baseline: 850.2395x; 850.2395x over previous
"""Trainium2 Bass kernel for nn_EuclidDistance_Assign_Module (vq_codebook).

LayerNorm -> euclidean distance to 256 codebook vectors -> argmin labels,
softmax(-32*d) assignment, assignment @ codebook reconstruction, plus
codebook self-distance matrix.

Data-parallel over batch B=8 across 8 NeuronCores. Each core handles
16384 tokens (C=512, K=256) independently; cluster_center is replicated.

Self-contained: hardcodes shapes from the problem spec.
"""

import os
import sys

import numpy as np

sys.path.insert(0, "/opt/trn_rl_repo")

import json  # noqa: E402
import types  # noqa: E402

import concourse.bass as bass  # noqa: E402
import concourse.tile as tile  # noqa: E402
from concourse import masks, mybir  # noqa: E402
from concourse.bass_utils import run_bass_kernel_spmd  # noqa: E402


def _split_excess_waits(bir: bytes, max_waits: int = 1) -> bytes:
    """This container's walrus build allows only one sync-wait per
    instruction; Tile emits several. Split the excess onto Drain carriers
    inserted just before the over-waiting instruction (same engine, so the
    waits still complete before it issues)."""
    d = json.loads(bir)
    for f in d["functions"]:
        for bb in f["blocks"]:
            new_insts = []
            ctr = 0
            for inst in bb["instructions"]:
                si = inst.get("sync_info") or {}
                ow = si.get("on_wait") or []
                if len(ow) > max_waits:
                    excess, keep = ow[:-max_waits], ow[-max_waits:]
                    for i in range(0, len(excess), max_waits):
                        ctr += 1
                        new_insts.append({
                            "name": f"{inst['name']}-ws{ctr}",
                            "opcode": "Drain",
                            "engine": inst["engine"],
                            "debug": inst.get("debug", 0),
                            "ins": [], "outs": [],
                            "sync_info": {"on_update": [],
                                          "on_wait": excess[i:i + max_waits]},
                        })
                    si["on_wait"] = keep
                    inst["sync_info"] = si
                new_insts.append(inst)
            bb["instructions"] = new_insts
    return json.dumps(d).encode()

F32 = mybir.dt.float32
I32 = mybir.dt.int32
AF = mybir.ActivationFunctionType
ALU = mybir.AluOpType
AX = mybir.AxisListType

N_CORES = 8
P = 128            # partitions / tokens per tile
C = 512            # channels
K = 256            # codebook size
N_TOK = 16384      # tokens per core (= 16*32*32)
NT = N_TOK // P    # 128 tiles per core
ALPHA = 32.0
LN_EPS = 1e-5

_CACHE = {}


def _build_program():
    nc = bass.Bass("TRN2", target_bir_lowering=False, debug=False)

    x_d = nc.dram_tensor("x", [N_TOK, C], F32, kind="ExternalInput").ap()
    cc_d = nc.dram_tensor("cluster_center", [K, C], F32, kind="ExternalInput").ap()

    dist_d = nc.dram_tensor("x_distance", [N_TOK, K], F32, kind="ExternalOutput").ap()
    asgn_d = nc.dram_tensor("x_assign", [N_TOK, K], F32, kind="ExternalOutput").ap()
    cdist_d = nc.dram_tensor("cluster_dist", [K, K], F32, kind="ExternalOutput").ap()
    xrec_d = nc.dram_tensor("x_rec", [N_TOK, C], F32, kind="ExternalOutput").ap()
    feat_d = nc.dram_tensor("feature", [N_TOK, C], F32, kind="ExternalOutput").ap()
    lbl_d = nc.dram_tensor("feature_label", [N_TOK], I32, kind="ExternalOutput").ap()

    from contextlib import ExitStack

    with tile.TileContext(nc) as tc:
        with ExitStack() as ctx:
            const = ctx.enter_context(tc.tile_pool(name="const", bufs=1))
            psum = ctx.enter_context(tc.tile_pool(name="psum", bufs=2, space="PSUM"))
            p_big = ctx.enter_context(tc.tile_pool(name="big", bufs=4))
            p_mid = ctx.enter_context(tc.tile_pool(name="mid", bufs=4))
            p_sm = ctx.enter_context(tc.tile_pool(name="small", bufs=6))

            # ---------------- constants ----------------
            ident = const.tile([P, P], F32)
            masks.make_identity(nc, ident[:])

            ones1 = const.tile([1, P], F32)
            nc.gpsimd.memset(ones1[:], 1.0)

            eps_t = const.tile([P, 1], F32)
            nc.gpsimd.memset(eps_t[:], LN_EPS)

            # codebook rows: 2 tiles [128, 512]
            cc_sb = []
            for m in range(2):
                t = const.tile([P, C], F32, tag=f"cc{m}")
                nc.sync.dma_start(t[:], cc_d[m * P:(m + 1) * P, :])
                cc_sb.append(t)

            # |c_k|^2 as column pair [128, 2] (chunk m in column m)
            c2cp = const.tile([P, 2], F32)
            for m in range(2):
                ccsq = p_big.tile([P, C], F32, tag="xn")
                nc.gpsimd.tensor_tensor(
                    ccsq[:], cc_sb[m][:], cc_sb[m][:], op=ALU.mult,
                )
                nc.vector.tensor_reduce(
                    c2cp[:, m:m + 1], ccsq[:], axis=AX.X, op=ALU.add,
                )

            # transpose to get |c|^2 as a [1, 256] row
            c2t_ps = psum.tile([2, P], F32, tag="mps")
            nc.tensor.transpose(c2t_ps[:], c2cp[:], ident[:])
            c2pt = const.tile([2, P], F32)
            nc.scalar.copy(c2pt[:], c2t_ps[:])
            c2row = const.tile([1, K], F32)
            nc.sync.dma_start(c2row[0:1, 0:P], c2pt[0:1, :])
            nc.sync.dma_start(c2row[0:1, P:K], c2pt[1:2, :])
            # -2 * |c|^2 row for the cluster_dist pass
            c2rowm2 = const.tile([1, K], F32)
            nc.vector.tensor_scalar_mul(c2rowm2[:], c2row[:], -2.0)

            # cT2[j] = -2 * cluster_center^T chunk j : [128(c), 256(k)]
            cT2_sb = []
            for j in range(4):
                ps = psum.tile([P, K], F32, tag="mps")
                for m in range(2):
                    nc.tensor.transpose(
                        ps[:, m * P:(m + 1) * P],
                        cc_sb[m][:, j * P:(j + 1) * P],
                        ident[:],
                    )
                t = const.tile([P, K], F32, tag=f"cT2_{j}")
                nc.scalar.mul(t[:], ps[:], -2.0)
                cT2_sb.append(t)

            # argmin helper: bigmidx[p, k] = 256 - k
            bigmidx = const.tile([P, K], F32)
            nc.gpsimd.iota(
                bigmidx[:], pattern=[[-1, K]], base=K, channel_multiplier=0,
                allow_small_or_imprecise_dtypes=True,
            )

            # per-tile labels parked here (col t), transposed+cast at the end
            lblbuf = const.tile([P, NT], F32)

            # ---------------- cluster_dist [256,256] ----------------
            # dist2[k1,k2] = |c_k1|^2 + |c_k2|^2 - 2<c_k1,c_k2>
            # matmul of cT2 x cT2 gives 4*<,>, so rescale by -0.5 and use
            # the -2*|c|^2 row via a K=1 matmul.
            for r in range(2):
                mps = psum.tile([P, K], F32, tag="mps")
                for j in range(4):
                    nc.tensor.matmul(
                        mps[:], lhsT=cT2_sb[j][:, r * P:(r + 1) * P],
                        rhs=cT2_sb[j][:], start=(j == 0), stop=False,
                    )
                nc.tensor.matmul(
                    mps[:], lhsT=ones1[:], rhs=c2rowm2[:], start=False, stop=True,
                )
                # t = max(-0.5*mps + |c_k1|^2, tiny)   [tiny: diagonal can
                # cancel to a small negative; Ln needs > 0]
                tcd = p_mid.tile([P, K], F32, tag="dist")
                nc.vector.tensor_scalar(
                    tcd[:], mps[:], -0.5, c2cp[:, r:r + 1],
                    op0=ALU.mult, op1=ALU.add,
                )
                tcd2 = p_mid.tile([P, K], F32, tag="e")
                nc.vector.tensor_scalar_max(tcd2[:], tcd[:], 1e-12)
                lncd = p_mid.tile([P, K], F32, tag="lnd")
                nc.scalar.activation(lncd[:], tcd2[:], AF.Ln)
                dcd = p_mid.tile([P, K], F32, tag="asgn")
                nc.scalar.activation(dcd[:], lncd[:], AF.Exp, scale=0.5)
                nc.sync.dma_start(cdist_d[r * P:(r + 1) * P, :], dcd[:])

            # ---------------- main loop over token tiles ----------------
            for t in range(NT):
                xt = p_big.tile([P, C], F32, tag="x")
                nc.sync.dma_start(xt[:], x_d[t * P:(t + 1) * P, :])

                # LayerNorm stats
                st6 = p_sm.tile([P, 6], F32, tag="st6")
                nc.vector.bn_stats(st6[:], xt[:])
                mv = p_sm.tile([P, 2], F32, tag="mv")
                nc.vector.bn_aggr(mv[:], st6[:])
                mean = mv[:, 0:1]
                var = mv[:, 1:2]

                # rstd = exp(-0.5*ln(var+eps));  (Rsqrt ACT is banned)
                lnv = p_sm.tile([P, 1], F32, tag="lnv")
                nc.scalar.activation(lnv[:], var, AF.Ln, bias=eps_t[:])
                rstd = p_sm.tile([P, 1], F32, tag="rstd")
                nc.scalar.activation(rstd[:], lnv[:], AF.Exp, scale=-0.5)

                # |xn|^2 = 512 * var * rstd^2 (exact: sum((x-mu)^2)*rstd^2)
                xn2a = p_sm.tile([P, 1], F32, tag="xn2a")
                nc.gpsimd.tensor_scalar(
                    xn2a[:], rstd[:], rstd[:], float(C), op0=ALU.mult, op1=ALU.mult,
                )
                xn2 = p_sm.tile([P, 1], F32, tag="xn2")
                nc.gpsimd.tensor_scalar(
                    xn2[:], xn2a[:], var, None, op0=ALU.mult,
                )
                # -mu*rstd
                nmr = p_sm.tile([P, 1], F32, tag="nmr")
                nc.gpsimd.tensor_scalar(
                    nmr[:], mean, rstd[:], -1.0, op0=ALU.mult, op1=ALU.mult,
                )

                # xn = x*rstd + (-mu*rstd)   (gamma=1, beta=0)
                xn = p_big.tile([P, C], F32, tag="xn")
                nc.gpsimd.tensor_scalar(
                    xn[:], xt[:], rstd[:], nmr[:], op0=ALU.mult, op1=ALU.add,
                )
                nc.sync.dma_start(feat_d[t * P:(t + 1) * P, :], xn[:])

                # xn^T via PE transpose, then SBUF copy for matmul weights
                xnT_ps = psum.tile([P, C], F32, tag="xnT_ps")
                for j in range(4):
                    nc.tensor.transpose(
                        xnT_ps[:, j * P:(j + 1) * P], xn[:, j * P:(j + 1) * P],
                        ident[:],
                    )
                xnT = p_big.tile([P, C], F32, tag="xnT")
                nc.scalar.copy(xnT[:], xnT_ps[:])

                # M = -2*xn.c + |c|^2   (c2row folded in as K=1 matmul)
                mps = psum.tile([P, K], F32, tag="mps")
                for j in range(4):
                    nc.tensor.matmul(
                        mps[:], lhsT=xnT[:, j * P:(j + 1) * P], rhs=cT2_sb[j][:],
                        start=(j == 0), stop=False,
                    )
                nc.tensor.matmul(
                    mps[:], lhsT=ones1[:], rhs=c2row[:], start=False, stop=True,
                )

                # dist = exp(0.5*ln(M + |xn|^2)) = sqrt(dist^2)
                lnd = p_mid.tile([P, K], F32, tag="lnd")
                nc.scalar.activation(lnd[:], mps[:], AF.Ln, bias=xn2[:])
                dist = p_mid.tile([P, K], F32, tag="dist")
                nc.scalar.activation(dist[:], lnd[:], AF.Exp, scale=0.5)
                nc.sync.dma_start(dist_d[t * P:(t + 1) * P, :], dist[:])

                # softmax(-alpha*dist) over k
                dmin = p_sm.tile([P, 1], F32, tag="dmin")
                nc.vector.tensor_reduce(dmin[:], dist[:], axis=AX.X, op=ALU.min)
                nab = p_sm.tile([P, 1], F32, tag="nab")
                nc.gpsimd.tensor_scalar(nab[:], dmin[:], ALPHA, None, op0=ALU.mult)
                e = p_mid.tile([P, K], F32, tag="e")
                esum = p_sm.tile([P, 1], F32, tag="esum")
                nc.scalar.activation(
                    e[:], dist[:], AF.Exp, bias=nab[:], scale=-ALPHA,
                    accum_out=esum[:],
                )
                res = p_sm.tile([P, 1], F32, tag="res")
                nc.vector.reciprocal(res[:], esum[:])
                asgn = p_mid.tile([P, K], F32, tag="asgn")
                nc.gpsimd.tensor_scalar(asgn[:], e[:], res[:], None, op0=ALU.mult)
                nc.sync.dma_start(asgn_d[t * P:(t + 1) * P, :], asgn[:])

                # argmin: first k where dist == dmin
                # junk = (dist == dmin) * (K - k); max over k = K - argmin
                eq = p_mid.tile([P, K], F32, tag="eq")
                nc.gpsimd.tensor_scalar(
                    eq[:], dist[:], dmin[:], None, op0=ALU.is_equal,
                )
                junk = p_mid.tile([P, K], F32, tag="junk")
                nc.gpsimd.tensor_tensor(junk[:], eq[:], bigmidx[:], op=ALU.mult)
                mx = p_sm.tile([P, 1], F32, tag="mx")
                nc.vector.tensor_reduce(mx[:], junk[:], axis=AX.X, op=ALU.max)
                nc.gpsimd.tensor_scalar(
                    lblbuf[:, t:t + 1], mx[:], -1.0, float(K),
                    op0=ALU.mult, op1=ALU.add,
                )

                # x_rec = asgn @ cc
                aT_ps = psum.tile([P, K], F32, tag="aT_ps")
                for m in range(2):
                    nc.tensor.transpose(
                        aT_ps[:, m * P:(m + 1) * P], asgn[:, m * P:(m + 1) * P],
                        ident[:],
                    )
                aT = p_mid.tile([P, K], F32, tag="aT")
                nc.vector.tensor_copy(aT[:], aT_ps[:])
                xr_ps = psum.tile([P, C], F32, tag="xr_ps")
                for m in range(2):
                    nc.tensor.matmul(
                        xr_ps[:], lhsT=aT[:, m * P:(m + 1) * P], rhs=cc_sb[m][:],
                        start=(m == 0), stop=(m == 1),
                    )
                xr = p_big.tile([P, C], F32, tag="xr")
                nc.vector.tensor_copy(xr[:], xr_ps[:])
                nc.sync.dma_start(xrec_d[t * P:(t + 1) * P, :], xr[:])

            # ---------------- labels: transpose + cast + store ----------------
            lT_ps = psum.tile([NT, P], F32, tag="xnT_ps")
            nc.tensor.transpose(lT_ps[:], lblbuf[:], ident[:])
            lT = p_big.tile([NT, P], I32, tag="lT")
            nc.scalar.copy(lT[:], lT_ps[:])
            nc.sync.dma_start(
                lbl_d.rearrange("(a b) -> a b", b=P)[:], lT[:],
            )

    orig_to_json = nc.to_json_bytes
    nc.to_json_bytes = types.MethodType(
        lambda self: _split_excess_waits(orig_to_json()), nc
    )
    return nc


def _run_numpy_fallback(x, cluster_center, ln_gamma, ln_beta):
    """General-input reference path (used only if LN params aren't 1/0)."""
    B, D, H, W, Cc = x.shape
    Kk = cluster_center.shape[0]
    xf = x.astype(np.float32)
    mu = xf.mean(-1, keepdims=True)
    var = ((xf - mu) ** 2).mean(-1, keepdims=True)
    xn = (xf - mu) / np.sqrt(var + LN_EPS) * ln_gamma + ln_beta
    x_re = xn.reshape(B, D * H * W, Cc)
    cc = cluster_center.astype(np.float32)

    def euclid(a, b):
        sq = (np.sum(a * a, -1, keepdims=True)
              + np.sum(b * b, -1)[None, :]
              - 2.0 * (a @ b.T))
        return np.sqrt(np.maximum(sq, 0.0))

    dist = np.stack([euclid(x_re[b], cc) for b in range(B)])
    feature_label = dist.argmin(-1).reshape(-1).astype(np.int32)
    x_distance = dist.reshape(B, D, H, W, Kk)
    z = -ALPHA * x_distance
    z = z - z.max(-1, keepdims=True)
    ez = np.exp(z)
    x_distance_assign = ez / ez.sum(-1, keepdims=True)
    cluster_dist = euclid(cc, cc)
    x_rec = np.einsum('bdhwk,kc->bdhwc', x_distance_assign, cc)
    feature = x_re.reshape(-1, Cc)
    return (x_distance, x_distance_assign, cluster_dist, x_rec,
            feature, feature_label)


LAST_EXEC_NS = None


class _Runner:
    """Compile the bass program once into a reusable sharded jit callable
    (mirrors concourse.bass2jax.run_bass_via_pjrt's multi-core path)."""

    def __init__(self):
        import jax
        from jax.sharding import Mesh, PartitionSpec
        from jax.experimental.shard_map import shard_map
        from concourse import bass2jax

        self.jax = jax
        nc = _build_program()
        bass2jax.install_neuronx_cc_hook()

        partition_name = (nc.partition_id_tensor.name
                          if nc.partition_id_tensor else None)
        in_names: list[str] = []
        out_names: list[str] = []
        out_avals = []
        for alloc in nc.m.functions[0].allocations:
            if not isinstance(alloc, mybir.MemoryLocationSet):
                continue
            name = alloc.memorylocations[0].name
            if alloc.kind == "ExternalInput":
                if name != partition_name:
                    in_names.append(name)
            elif alloc.kind == "ExternalOutput":
                out_names.append(name)
                out_avals.append(jax.core.ShapedArray(
                    tuple(alloc.tensor_shape), mybir.dt.np(alloc.dtype)))
        self.in_names = list(in_names)
        self.out_names = out_names
        self.out_avals = out_avals
        n_params = len(in_names)
        n_outs = len(out_avals)
        all_in_names = in_names + out_names
        if partition_name is not None:
            all_in_names = all_in_names + [partition_name]

        def _body(*args):
            operands = list(args)
            if partition_name is not None:
                operands.append(bass2jax.partition_id_tensor())
            outs = bass2jax._bass_exec_p.bind(
                *operands,
                out_avals=tuple(out_avals),
                in_names=tuple(all_in_names),
                out_names=tuple(out_names),
                lowering_input_output_aliases=(),
                sim_require_finite=True,
                sim_require_nnan=True,
                nc=nc,
            )
            return tuple(outs)

        devices = jax.devices()[:N_CORES]
        self.mesh = Mesh(np.asarray(devices), ("core",))
        self.pspec = PartitionSpec("core")
        in_specs = (self.pspec,) * (n_params + n_outs)
        out_specs = (self.pspec,) * n_outs
        donate = tuple(range(n_params, n_params + n_outs))
        self.sharded = jax.jit(
            shard_map(_body, mesh=self.mesh, in_specs=in_specs,
                      out_specs=out_specs, check_rep=False),
            donate_argnums=donate, keep_unused=True,
        )

    def _concat_inputs(self, in_maps):
        return [
            np.concatenate([np.asarray(m[name]) for m in in_maps], axis=0)
            for name in self.in_names
        ]

    def _zeros(self):
        return [
            np.zeros((N_CORES * a.shape[0], *a.shape[1:]), a.dtype)
            for a in self.out_avals
        ]

    def run(self, in_maps):
        out_arrs = self.sharded(*self._concat_inputs(in_maps), *self._zeros())
        return [
            {name: np.asarray(out_arrs[i]).reshape(
                N_CORES, *self.out_avals[i].shape)[c]
             for i, name in enumerate(self.out_names)}
            for c in range(N_CORES)
        ]

    def time_exec(self, in_maps, iters=5):
        """Min wall time of the sharded executable with inputs staged on
        device and outputs left on device (excludes host<->device copies;
        includes dispatch overhead)."""
        import time as _time
        jax = self.jax
        from jax.sharding import NamedSharding
        sh = NamedSharding(self.mesh, self.pspec)
        din = [jax.device_put(a, sh) for a in self._concat_inputs(in_maps)]
        jax.block_until_ready(din)
        times = []
        out = None
        for _ in range(iters):
            dz = [jax.device_put(z, sh) for z in self._zeros()]
            jax.block_until_ready(dz)
            t0 = _time.perf_counter()
            out = self.sharded(*din, *dz)
            jax.block_until_ready(out)
            times.append(_time.perf_counter() - t0)
        return times, out


def _get_runner():
    if "runner" not in _CACHE:
        _CACHE["runner"] = _Runner()
    return _CACHE["runner"]


def _make_in_maps(x, cluster_center):
    B = x.shape[0]
    x_flat = np.ascontiguousarray(x.reshape(B, N_TOK, C))
    return [
        {"x": x_flat[b], "cluster_center": cluster_center} for b in range(B)
    ]


def kernel(x, cluster_center, ln_gamma, ln_beta):
    x = np.asarray(x, dtype=np.float32)
    cluster_center = np.asarray(cluster_center, dtype=np.float32)
    ln_gamma = np.asarray(ln_gamma, dtype=np.float32)
    ln_beta = np.asarray(ln_beta, dtype=np.float32)

    if (x.shape != (8, 16, 32, 32, 512) or cluster_center.shape != (256, 512)
            or not np.all(ln_gamma == 1.0) or not np.all(ln_beta == 0.0)):
        return _run_numpy_fallback(x, cluster_center, ln_gamma, ln_beta)

    res = _get_runner().run(_make_in_maps(x, cluster_center))

    B = x.shape[0]
    x_distance = np.stack([res[b]["x_distance"] for b in range(B)])
    x_distance = x_distance.reshape(8, 16, 32, 32, K)
    x_assign = np.stack([res[b]["x_assign"] for b in range(B)])
    x_assign = x_assign.reshape(8, 16, 32, 32, K)
    cluster_dist = res[0]["cluster_dist"]
    x_rec = np.stack([res[b]["x_rec"] for b in range(B)])
    x_rec = x_rec.reshape(8, 16, 32, 32, C)
    feature = np.concatenate([res[b]["feature"] for b in range(B)], axis=0)
    feature_label = np.concatenate(
        [res[b]["feature_label"] for b in range(B)], axis=0
    ).astype(np.int32)

    return (x_distance, x_assign, cluster_dist, x_rec, feature, feature_label)
